# revision 1
# baseline (speedup 1.0000x reference)
"""GATNet on 8 Trainium2 NeuronCores (Bass/Tile, SPMD).

Strategy: sort edges (+self loops) by dst on host; shard dst nodes 8 ways
(2048 dst/core, 16 blocks of 128 dst). Per 128-edge chunk: dma_gather 512B
x-rows, recompute h = x @ W1aug on TensorE (16x less DMA than gathering
h-rows), and do the softmax-weighted neighbor aggregation as one-hot-mask
matmuls accumulated in PSUM. Attention logits ride along as extra matmul
columns (es = x @ (W1 @ a_src) folded into W1aug; linear bias via a ones
column of x). Softmax without max-subtraction (logits are O(1); exactly
invariant). ELU materialized as elu+1 = min(exp(z),1) + relu(z) with the
-1 folded into downstream constants. Layer 2 gathers from an AllGathered
192-col table [h2 | es2 | ed2 | pad]. Graph max-pool on the transposed
output; the cell MLP (per-core 32-graph slice) overlaps the AllGather.
"""
import os
import numpy as np
from contextlib import ExitStack

import concourse.bacc as bacc
import concourse.tile as tile
import concourse.mybir as mybir
from concourse.bass_utils import run_bass_kernel_spmd

N, E, B = 16384, 131072, 256
F_IN, HID, HEADS, F_CELL, N_OUT = 78, 128, 10, 954, 2
NEG = 0.2
NCORES = 8
DPC = N // NCORES          # dst per core (2048)
BLK = 128
NBLK = DPC // BLK          # 16
GPC = B // NCORES          # graphs per core (32)
NPG = N // B               # nodes per graph (64)
F32 = mybir.dt.float32
I16 = mybir.dt.int16
AF = mybir.ActivationFunctionType
ALU = mybir.AluOpType

_CACHE = {}
_PHASE = int(os.environ.get("GAT_PHASE", "4"))
_DEBUG = int(os.environ.get("GAT_DEBUG", "0"))
_NBLKRUN = int(os.environ.get("GAT_BLOCKS", str(NBLK)))


# --------------------------------------------------------------------------
# host-side prep
# --------------------------------------------------------------------------

def _pack_idx(v, totch):
    """idx list [totch*128] -> [128, totch*8] int16 wrapped-16, replicated."""
    a = v.reshape(totch * 8, 16).T.astype(np.int16)
    return np.ascontiguousarray(np.tile(a, (8, 1)))


def _prep(inputs):
    f32 = lambda k: np.asarray(inputs[k], np.float32)
    x, W1, b1 = f32("x"), f32("W1"), f32("b1")
    a_s1, a_d1 = f32("a_src1"), f32("a_dst1")
    W2, a_s2, a_d2, b2 = f32("W2"), f32("a_src2"), f32("a_dst2"), f32("b2")
    Wg, bg, cell = f32("Wg"), f32("bg"), f32("cell")
    Wf1, bf1 = f32("Wf1"), f32("bf1")
    Wf2, bf2 = f32("Wf2"), f32("bf2")
    Wf3, bf3 = f32("Wf3"), f32("bf3")
    Wo, bo = f32("Wo"), f32("bo")
    ei = np.asarray(inputs["edge_index"], np.int64)

    src = np.concatenate([ei[0], np.arange(N, dtype=np.int64)])
    dst = np.concatenate([ei[1], np.arange(N, dtype=np.int64)])
    order = np.argsort(dst, kind="stable")
    src, dst = src[order], dst[order]
    gblk = dst // BLK
    starts = np.searchsorted(gblk, np.arange(N // BLK))
    ends = np.searchsorted(gblk, np.arange(N // BLK) + 1)

    M_list = []
    for slot in range(NBLK):
        mx = max(int(ends[c * NBLK + slot] - starts[c * NBLK + slot])
                 for c in range(NCORES))
        M_list.append(max(1, (mx + 127) // 128))
    totch = int(sum(M_list))

    iota = np.arange(128, dtype=np.float32)
    per_core = []
    for c in range(NCORES):
        srcs = np.zeros(totch * 128, np.int64)
        dsts = np.zeros(totch * 128, np.int64)
        dloc = np.full(totch * 128, -1.0, np.float32)
        off = 0
        for slot in range(NBLK):
            g = c * NBLK + slot
            s0, s1 = int(starts[g]), int(ends[g])
            n = s1 - s0
            srcs[off:off + n] = src[s0:s1]
            dsts[off:off + n] = dst[s0:s1]
            dloc[off:off + n] = (dst[s0:s1] - g * BLK).astype(np.float32)
            off += M_list[slot] * 128
        dl = dloc.reshape(totch, 128)
        mm = (dl[:, :, None] == iota[None, None, :]).astype(np.float32)
        mt = (dl[:, None, :] == iota[None, :, None]).astype(np.float32)
        per_core.append(dict(
            idx1=_pack_idx(srcs, totch),
            idxd2=_pack_idx(dsts, totch),
            mm=np.ascontiguousarray(
                mm.transpose(1, 0, 2).reshape(128, totch * 128)),
            mt=np.ascontiguousarray(
                mt.transpose(1, 0, 2).reshape(128, totch * 128)),
        ))

    W1aug = np.zeros((80, 1312), np.float32)
    W1aug[:F_IN, :1280] = W1
    W1aug[F_IN, :1280] = b1
    W1aug[:F_IN, 1280:1290] = np.einsum(
        "khc,hc->kh", W1.reshape(F_IN, HEADS, HID), a_s1)
    A_dst = np.zeros((80, 16), np.float32)
    A_dst[:F_IN, :HEADS] = np.einsum(
        "khc,hc->kh", W1.reshape(F_IN, HEADS, HID), a_d1)

    A2s = (W2 @ a_s2[0]).astype(np.float32)
    A2d = (W2 @ a_d2[0]).astype(np.float32)
    W2aug = np.zeros((1280, 132), np.float32)
    W2aug[:, :HID] = W2
    W2aug[:, HID] = A2s
    W2aug[:, HID + 1] = A2d
    b2mod = np.zeros((128, 132), np.float32)
    b2mod[:, :HID] = (b2 - W2.sum(axis=0))[None, :]
    sh2v = -(A2s.sum() + A2d.sum())
    shift2 = np.zeros((128, 2), np.float32)
    shift2[:, 0] = sh2v
    shift2[:, 1] = NEG * sh2v

    xpad = np.zeros((N, 128), np.float32)
    xpad[:, :F_IN] = x
    xpad[:, F_IN] = 1.0

    bgmod = np.ascontiguousarray(
        np.tile((bg - Wg.sum(axis=0))[None, :], (GPC, 1))).astype(np.float32)
    ident = np.eye(128, dtype=np.float32)

    Wf1p = np.zeros((1024, 2048), np.float32)
    Wf1p[:F_CELL] = Wf1
    bobc = np.ascontiguousarray(np.tile(bo[None, :], (GPC, 1))).astype(
        np.float32)
    bf1c = np.ascontiguousarray(bf1.reshape(16, 128).T)
    bf2c = np.ascontiguousarray(bf2.reshape(4, 128).T)
    bf3c = np.ascontiguousarray(bf3.reshape(1, 128).T)

    shared = dict(
        xpad=xpad, W1aug=W1aug, A_dst=A_dst, W2aug=W2aug, b2mod=b2mod,
        shift2=shift2, Wg=Wg, bgmod=bgmod, ident=ident,
        Wf1=Wf1p, Wf2=Wf2, Wf3=Wf3, Wo=Wo, bobc=bobc,
        bf1c=bf1c, bf2c=bf2c, bf3c=bf3c,
    )
    in_maps = []
    for c in range(NCORES):
        m = dict(shared)
        m.update(per_core[c])
        m["xown"] = np.ascontiguousarray(xpad[c * DPC:(c + 1) * DPC])
        cT = np.zeros((1024, GPC), np.float32)
        cT[:F_CELL] = cell[c * GPC:(c + 1) * GPC].T
        m["cellT"] = cT
        in_maps.append(m)
    return tuple(M_list), in_maps


# --------------------------------------------------------------------------
# device program
# --------------------------------------------------------------------------

def _build(M_list):
    M_list = list(M_list)
    totch = sum(M_list)
    maxM = max(M_list)
    nc = bacc.Bacc("TRN2", target_bir_lowering=False, debug=False,
                   num_devices=NCORES)

    def din(name, shape, dt=F32):
        return nc.dram_tensor(name, shape, dt, kind="ExternalInput").ap()

    xpad = din("xpad", [N, 128])
    xown = din("xown", [DPC, 128])
    idx1 = din("idx1", [128, totch * 8], I16)
    idxd2 = din("idxd2", [128, totch * 8], I16)
    mm_d = din("mm", [128, totch * 128])
    mt_d = din("mt", [128, totch * 128])
    W1aug = din("W1aug", [80, 1312])
    A_dst = din("A_dst", [80, 16])
    W2aug = din("W2aug", [1280, 132])
    b2mod = din("b2mod", [128, 132])
    shift2 = din("shift2", [128, 2])
    Wg_d = din("Wg", [128, 128])
    bgmod = din("bgmod", [GPC, 128])
    ident = din("ident", [128, 128])
    Wf1 = din("Wf1", [1024, 2048])
    Wf2 = din("Wf2", [2048, 512])
    Wf3 = din("Wf3", [512, 128])
    Wo_d = din("Wo", [128, N_OUT])
    bobc = din("bobc", [GPC, N_OUT])
    cellT = din("cellT", [1024, GPC])
    bf1c = din("bf1c", [128, 16])
    bf2c = din("bf2c", [128, 4])
    bf3c = din("bf3c", [128, 1])

    out_d = nc.dram_tensor("out", [GPC, 130], F32, kind="ExternalOutput").ap()
    dbg_d = None
    if _DEBUG:
        dbg_d = nc.dram_tensor("dbg", [DPC, 192], F32,
                               kind="ExternalOutput").ap()

    ag_in = nc.dram_tensor("ag_in", [DPC, 192], F32)
    ag_out = nc.dram_tensor("ag_out", [N, 192], F32, addr_space="Shared")

    with tile.TileContext(nc) as tc, ExitStack() as ctx:
        cst = ctx.enter_context(tc.tile_pool(name="cst", bufs=1))
        big = ctx.enter_context(tc.tile_pool(name="big", bufs=1))
        g1p = ctx.enter_context(tc.tile_pool(name="g1p", bufs=2))
        mmp = ctx.enter_context(tc.tile_pool(name="mmp", bufs=3))
        wbp = ctx.enter_context(tc.tile_pool(name="wbp", bufs=2))
        sml = ctx.enter_context(tc.tile_pool(name="sml", bufs=3))
        evp = ctx.enter_context(tc.tile_pool(name="evp", bufs=2))
        ps_tr = ctx.enter_context(
            tc.tile_pool(name="ps_tr", bufs=2, space="PSUM"))
        ps_b = ctx.enter_context(
            tc.tile_pool(name="ps_b", bufs=1, space="PSUM"))
        ps_o = ctx.enter_context(
            tc.tile_pool(name="ps_o", bufs=1, space="PSUM"))

        # ---- constants ----
        t_w1 = cst.tile([80, 1312], F32)
        nc.scalar.dma_start(t_w1[:], W1aug)
        t_ad = cst.tile([80, 16], F32)
        nc.scalar.dma_start(t_ad[:], A_dst)
        t_id = cst.tile([128, 128], F32)
        nc.scalar.dma_start(t_id[:], ident)
        t_idx1 = cst.tile([128, totch * 8], I16)
        nc.scalar.dma_start(t_idx1[:], idx1)
        t_idxd2 = cst.tile([128, totch * 8], I16)
        nc.scalar.dma_start(t_idxd2[:], idxd2)
        t_w2 = cst.tile([128, 10, 132], F32)
        nc.scalar.dma_start(
            t_w2[:], W2aug.rearrange("(a p) c -> p a c", p=128))
        t_b2m = cst.tile([128, 132], F32)
        nc.scalar.dma_start(t_b2m[:], b2mod)
        t_sh2 = cst.tile([128, 2], F32)
        nc.scalar.dma_start(t_sh2[:], shift2)
        t_wg = cst.tile([128, 128], F32)
        nc.scalar.dma_start(t_wg[:], Wg_d)
        t_bgm = cst.tile([GPC, 128], F32)
        nc.scalar.dma_start(t_bgm[:], bgmod)
        t_wo = cst.tile([128, N_OUT], F32)
        nc.scalar.dma_start(t_wo[:], Wo_d)
        t_bo = cst.tile([GPC, N_OUT], F32)
        nc.scalar.dma_start(t_bo[:], bobc)
        t_bf1 = cst.tile([128, 16], F32)
        nc.scalar.dma_start(t_bf1[:], bf1c)
        t_bf2 = cst.tile([128, 4], F32)
        nc.scalar.dma_start(t_bf2[:], bf2c)
        t_bf3 = cst.tile([128, 1], F32)
        nc.scalar.dma_start(t_bf3[:], bf3c)
        t_ones = cst.tile([128, 1], F32)
        nc.vector.memset(t_ones[:], 1.0)
        t_onesr = cst.tile([1, 128], F32)
        nc.vector.memset(t_onesr[:], 1.0)

        # persistent activations
        x1yT = big.tile([128, 10, NBLK, 128], F32)   # [c, k, blk, d]
        x2yT = big.tile([128, DPC], F32)
        t_osb = big.tile([GPC, 130], F32)
        nc.vector.memset(t_osb[:], 0.0)

        # ==================== layer 1 ====================
        off = 0
        for blk in range(_NBLKRUN):
            nch = M_list[blk]
            # ed_blk = x_blk @ A_dst  via transpose + matmul
            t_xb = sml.tile([128, 128], F32, tag="xb")
            nc.scalar.dma_start(t_xb[:], xown[blk * 128:(blk + 1) * 128, :])
            p_xbt = ps_tr.tile([128, 128], F32, tag="tr")
            nc.tensor.transpose(p_xbt[:], t_xb[:], t_id[:])
            t_xbt = sml.tile([128, 128], F32, tag="xbt")
            nc.vector.tensor_copy(t_xbt[:], p_xbt[:])
            p_ed = ps_tr.tile([128, 128], F32, tag="tr")
            nc.tensor.matmul(p_ed[0:128, 0:10], t_xbt[0:80, :],
                             t_ad[:, 0:10], start=True, stop=True)
            t_edb = sml.tile([128, 16], F32, tag="edb")
            nc.vector.tensor_copy(t_edb[:, 0:10], p_ed[0:128, 0:10])

            # gather this block's src x-rows
            t_g = g1p.tile([128, maxM, 128], F32, tag="g1")
            nidx = nch * 128
            nc.gpsimd.dma_gather(
                t_g[:, 0:nch, :], xpad,
                t_idx1[:, off * 8:(off + nch) * 8], nidx, nidx, 128,
                single_packet=False)

            p_out = ps_o.tile([128, 1536], F32, tag="acc")
            for ch in range(nch):
                p_t = ps_tr.tile([128, 128], F32, tag="tr")
                nc.tensor.transpose(p_t[:], t_g[:, ch, :], t_id[:])
                t_xgt = sml.tile([128, 128], F32, tag="xgt")
                nc.scalar.copy(t_xgt[:], p_t[:])

                # B_aug = xg @ [W1 | b1-row | A_src]; edg accumulates into
                # the es columns so es+edg is already summed in PSUM
                p_b = ps_b.tile([128, 1536], F32, tag="b")
                nc.tensor.matmul(p_b[:, 0:512], t_xgt[0:80, :],
                                 t_w1[:, 0:512], start=True, stop=True)
                nc.tensor.matmul(p_b[:, 512:1024], t_xgt[0:80, :],
                                 t_w1[:, 512:1024], start=True, stop=True)
                nc.tensor.matmul(p_b[:, 1024:1280], t_xgt[0:80, :],
                                 t_w1[:, 1024:1280], start=True, stop=True)
                # es region: own accumulation group (es then +edg)
                nc.tensor.matmul(p_b[:, 1280:1290], t_xgt[0:80, :],
                                 t_w1[:, 1280:1290], start=True, stop=False)
                t_mt = mmp.tile([128, 128], F32, tag="mt")
                nc.scalar.dma_start(
                    t_mt[:], mt_d[:, (off + ch) * 128:(off + ch + 1) * 128])
                nc.tensor.matmul(p_b[:, 1280:1290], t_mt[:],
                                 t_edb[:, 0:10], start=False, stop=True)

                # attention weights: lrelu(z) = max(z, 0.2 z)
                t_l2 = sml.tile([128, 10], F32, tag="tl2")
                nc.scalar.activation(t_l2[:], p_b[:, 1280:1290], AF.Copy,
                                     scale=NEG)
                t_lr = sml.tile([128, 10], F32, tag="tlr")
                nc.vector.tensor_tensor(t_lr[:], p_b[:, 1280:1290], t_l2[:],
                                        ALU.max)
                t_wv = sml.tile([128, 16], F32, tag="twv")
                nc.scalar.activation(t_wv[:, 0:10], t_lr[:], AF.Exp)

                # wB: per-head scaled B + w columns
                t_wb = wbp.tile([128, 1312], F32, tag="wb")
                for h in range(HEADS):
                    sl = slice(h * 128, (h + 1) * 128)
                    if h % 2 == 0:
                        nc.scalar.activation(
                            t_wb[:, sl], p_b[:, sl], AF.Copy,
                            scale=t_wv[:, h:h + 1])
                    else:
                        nc.vector.tensor_scalar_mul(
                            t_wb[:, sl], p_b[:, sl], t_wv[:, h:h + 1])
                nc.vector.tensor_copy(t_wb[:, 1280:1290], t_wv[:, 0:10])

                # masked scatter-accumulate
                t_mm = mmp.tile([128, 128], F32, tag="mm")
                nc.scalar.dma_start(
                    t_mm[:], mm_d[:, (off + ch) * 128:(off + ch + 1) * 128])
                st, sp = ch == 0, ch == nch - 1
                nc.tensor.matmul(p_out[:, 0:512], t_mm[:], t_wb[:, 0:512],
                                 start=st, stop=sp)
                nc.tensor.matmul(p_out[:, 512:1024], t_mm[:],
                                 t_wb[:, 512:1024], start=st, stop=sp)
                nc.tensor.matmul(p_out[:, 1024:1290], t_mm[:],
                                 t_wb[:, 1024:1290], start=st, stop=sp)

            # block evac: z = num/den ; y = min(exp z, 1) + relu z
            t_rc = sml.tile([128, 16], F32, tag="trc")
            nc.vector.reciprocal(t_rc[:, 0:10], p_out[:, 1280:1290])
            t_y = evp.tile([128, 1280], F32, tag="ty")
            t_u = evp.tile([128, 1280], F32, tag="tu")
            for h in range(HEADS):
                sl = slice(h * 128, (h + 1) * 128)
                nc.scalar.activation(t_u[:, sl], p_out[:, sl], AF.Exp,
                                     scale=t_rc[:, h:h + 1])
                nc.scalar.activation(t_y[:, sl], p_out[:, sl], AF.Relu,
                                     scale=t_rc[:, h:h + 1])
            nc.vector.tensor_scalar_min(t_u[:], t_u[:], 1.0)
            nc.vector.tensor_tensor(t_y[:], t_y[:], t_u[:], ALU.add)

            for k in range(10):
                p_t = ps_tr.tile([128, 128], F32, tag="tr")
                nc.tensor.transpose(
                    p_t[:], t_y[:, k * 128:(k + 1) * 128], t_id[:])
                if k % 2 == 0:
                    nc.scalar.copy(x1yT[:, k, blk, :], p_t[:])
                else:
                    nc.vector.tensor_copy(x1yT[:, k, blk, :], p_t[:])
            off += nch

        # ==================== h2 table + AllGather ====================
        for blk in range(_NBLKRUN):
            p_h2 = ps_o.tile([128, 1536], F32, tag="acc")
            for k in range(10):
                nc.tensor.matmul(p_h2[:, 0:132], x1yT[:, k, blk, :],
                                 t_w2[:, k, :], start=(k == 0), stop=(k == 9))
            t_h2 = evp.tile([128, 192], F32, tag="h2sb")
            nc.vector.memset(t_h2[:, 132:192], 0.0)
            nc.vector.tensor_tensor(t_h2[:, 0:132], p_h2[:, 0:132],
                                    t_b2m[:], ALU.add)
            nc.scalar.dma_start(ag_in.ap()[blk * 128:(blk + 1) * 128, :],
                                t_h2[:])

        if _DEBUG:
            t_dbg = evp.tile([128, 192], F32, tag="h2sb")
            for blk in range(NBLK):
                nc.scalar.dma_start(t_dbg[:],
                                    ag_in.ap()[blk * 128:(blk + 1) * 128, :])
                nc.scalar.dma_start(dbg_d[blk * 128:(blk + 1) * 128, :],
                                    t_dbg[:])

        if _PHASE >= 2:
            nc.gpsimd.collective_compute(
                "AllGather", ALU.bypass,
                replica_groups=[list(range(NCORES))],
                ins=[ag_in.ap().opt()],
                outs=[ag_out.ap().opt()],
            )

        # ==================== cell MLP (overlaps AG) ====================
        if _PHASE >= 3:
            t_cT = big.tile([128, 8, GPC], F32)
            nc.scalar.dma_start(
                t_cT[:], cellT.rearrange("(a p) g -> p a g", p=128))
            t_sq = evp.tile([128, 8 * GPC], F32, tag="csq")
            nc.vector.tensor_tensor(
                t_sq[:], t_cT[:].rearrange("p a g -> p (a g)"),
                t_cT[:].rearrange("p a g -> p (a g)"), ALU.mult)
            p_nrm = ps_tr.tile([128, 128], F32, tag="tr")
            for k in range(8):
                nc.tensor.matmul(p_nrm[0:1, 0:GPC], t_ones[:],
                                 t_sq[:, k * GPC:(k + 1) * GPC],
                                 start=(k == 0), stop=(k == 7))
            t_nr = sml.tile([1, GPC], F32, tag="tnr")
            nc.vector.tensor_scalar_max(t_nr[:], p_nrm[0:1, 0:GPC], 1e-24)
            t_nsq = sml.tile([1, GPC], F32, tag="tnsq")
            nc.scalar.activation(t_nsq[:], t_nr[:], AF.Sqrt)
            t_nrc = sml.tile([1, GPC], F32, tag="tnrc")
            nc.vector.reciprocal(t_nrc[:], t_nsq[:])
            p_rbc = ps_tr.tile([128, 128], F32, tag="tr")
            nc.tensor.matmul(p_rbc[0:128, 0:GPC], t_onesr[:], t_nrc[:],
                             start=True, stop=True)
            t_rbc = sml.tile([128, GPC], F32, tag="trbc")
            nc.vector.tensor_copy(t_rbc[:], p_rbc[0:128, 0:GPC])
            t_cn = big.tile([128, 8, GPC], F32)
            for k in range(8):
                nc.vector.tensor_tensor(t_cn[:, k, :], t_cT[:, k, :],
                                        t_rbc[:], ALU.mult)

            xc1T = big.tile([128, 16, GPC], F32)
            for c1 in range(16):
                p_f = ps_tr.tile([128, 128], F32, tag="tr")
                for k in range(8):
                    t_wf = sml.tile([128, 128], F32, tag="twf")
                    nc.scalar.dma_start(
                        t_wf[:], Wf1[k * 128:(k + 1) * 128,
                                     c1 * 128:(c1 + 1) * 128])
                    nc.tensor.matmul(p_f[0:128, 0:GPC], t_wf[:],
                                     t_cn[:, k, :],
                                     start=(k == 0), stop=(k == 7))
                nc.scalar.activation(xc1T[:, c1, :], p_f[0:128, 0:GPC],
                                     AF.Relu, bias=t_bf1[:, c1:c1 + 1])

            xc2T = big.tile([128, 4, GPC], F32)
            for c2 in range(4):
                p_f = ps_tr.tile([128, 128], F32, tag="tr")
                for k in range(16):
                    t_wf = sml.tile([128, 128], F32, tag="twf")
                    nc.scalar.dma_start(
                        t_wf[:], Wf2[k * 128:(k + 1) * 128,
                                     c2 * 128:(c2 + 1) * 128])
                    nc.tensor.matmul(p_f[0:128, 0:GPC], t_wf[:],
                                     xc1T[:, k, :],
                                     start=(k == 0), stop=(k == 15))
                nc.scalar.activation(xc2T[:, c2, :], p_f[0:128, 0:GPC],
                                     AF.Relu, bias=t_bf2[:, c2:c2 + 1])

            xc3T = big.tile([128, GPC], F32)
            p_f3 = ps_tr.tile([128, 128], F32, tag="tr")
            for k in range(4):
                t_wf = sml.tile([128, 128], F32, tag="twf")
                nc.scalar.dma_start(t_wf[:], Wf3[k * 128:(k + 1) * 128, :])
                nc.tensor.matmul(p_f3[0:128, 0:GPC], t_wf[:], xc2T[:, k, :],
                                 start=(k == 0), stop=(k == 3))
            nc.scalar.activation(xc3T[:], p_f3[0:128, 0:GPC], AF.Relu,
                                 bias=t_bf3[:, 0:1])

            p_oc = ps_tr.tile([128, 128], F32, tag="tr")
            nc.tensor.matmul(p_oc[0:GPC, 0:N_OUT], xc3T[:], t_wo[:],
                             start=True, stop=True)
            nc.vector.tensor_tensor(t_osb[:, 128:130], p_oc[0:GPC, 0:N_OUT],
                                    t_bo[:], ALU.add)

        # ==================== layer 2 ====================
        if _PHASE >= 4:
            off = 0
            for blk in range(NBLK):
                nch = M_list[blk]
                nidx = nch * 128
                t_g2 = g1p.tile([128, maxM, 192], F32, tag="g2")
                nc.gpsimd.dma_gather(
                    t_g2[:, 0:nch, :], ag_out.ap(),
                    t_idx1[:, off * 8:(off + nch) * 8], nidx, nidx, 192,
                    single_packet=False)
                t_gd = g1p.tile([128, maxM, 64], F32, tag="gd")
                nc.gpsimd.dma_gather(
                    t_gd[:, 0:nch, :], ag_out.ap()[:, 128:192],
                    t_idxd2[:, off * 8:(off + nch) * 8], nidx, nidx, 64,
                    elem_step=192, single_packet=False)

                p_out = ps_o.tile([128, 1536], F32, tag="acc")
                for ch in range(nch):
                    t_e = sml.tile([128, 1], F32, tag="te2")
                    nc.vector.tensor_tensor(
                        t_e[:], t_g2[:, ch, 128:129], t_gd[:, ch, 1:2],
                        ALU.add)
                    t_z = sml.tile([128, 1], F32, tag="tz2")
                    nc.scalar.activation(t_z[:], t_e[:], AF.Copy)
                    nc.vector.tensor_scalar_add(t_z[:], t_z[:],
                                                t_sh2[:, 0:1])
                    t_z2 = sml.tile([128, 1], F32, tag="tz22")
                    nc.scalar.activation(t_z2[:], t_e[:], AF.Copy, scale=NEG)
                    nc.vector.tensor_scalar_add(t_z2[:], t_z2[:],
                                                t_sh2[:, 1:2])
                    t_lr = sml.tile([128, 1], F32, tag="tlr2")
                    nc.vector.tensor_tensor(t_lr[:], t_z[:], t_z2[:],
                                            ALU.max)
                    t_w2v = sml.tile([128, 1], F32, tag="tw2")
                    nc.scalar.activation(t_w2v[:], t_lr[:], AF.Exp)

                    t_wh = wbp.tile([128, 132], F32, tag="wh2")
                    nc.scalar.activation(t_wh[:, 0:128], t_g2[:, ch, 0:128],
                                         AF.Copy, scale=t_w2v[:])
                    nc.vector.tensor_copy(t_wh[:, 128:129], t_w2v[:])

                    t_mm = mmp.tile([128, 128], F32, tag="mm")
                    nc.scalar.dma_start(
                        t_mm[:],
                        mm_d[:, (off + ch) * 128:(off + ch + 1) * 128])
                    nc.tensor.matmul(p_out[:, 0:129], t_mm[:],
                                     t_wh[:, 0:129],
                                     start=(ch == 0), stop=(ch == nch - 1))

                t_rc = sml.tile([128, 1], F32, tag="trc2")
                nc.vector.reciprocal(t_rc[:], p_out[:, 128:129])
                t_u = evp.tile([128, 128], F32, tag="tu2")
                t_y = evp.tile([128, 128], F32, tag="ty2")
                nc.scalar.activation(t_u[:], p_out[:, 0:128], AF.Exp,
                                     scale=t_rc[:])
                nc.scalar.activation(t_y[:], p_out[:, 0:128], AF.Relu,
                                     scale=t_rc[:])
                nc.vector.tensor_scalar_min(t_u[:], t_u[:], 1.0)
                nc.vector.tensor_tensor(t_y[:], t_y[:], t_u[:], ALU.add)
                p_t = ps_tr.tile([128, 128], F32, tag="tr")
                nc.tensor.transpose(p_t[:], t_y[:], t_id[:])
                nc.vector.tensor_copy(x2yT[:, blk * 128:(blk + 1) * 128],
                                      p_t[:])
                off += nch

            # ---- pool + graph head ----
            t_pool = sml.tile([128, GPC], F32, tag="pool")
            nc.vector.tensor_reduce(
                t_pool[:], x2yT[:].rearrange("p (g n) -> p g n", n=NPG),
                mybir.AxisListType.X, ALU.max)
            p_g1 = ps_tr.tile([128, 128], F32, tag="tr")
            nc.tensor.matmul(p_g1[0:GPC, 0:128], t_pool[:], t_wg[:],
                             start=True, stop=True)
            t_g1 = sml.tile([GPC, 128], F32, tag="tg1")
            nc.vector.tensor_tensor(t_g1[:], p_g1[0:GPC, 0:128], t_bgm[:],
                                    ALU.add)
            nc.scalar.activation(t_osb[:, 0:128], t_g1[:], AF.Relu)

        nc.scalar.dma_start(out_d, t_osb[:])

    nc.compile()
    return nc


# --------------------------------------------------------------------------
# entry point
# --------------------------------------------------------------------------

def kernel(**inputs):
    M_list, in_maps = _prep(inputs)
    if M_list not in _CACHE:
        _CACHE[M_list] = _build(M_list)
    nc = _CACHE[M_list]
    res = run_bass_kernel_spmd(nc, in_maps, list(range(NCORES)))
    out = np.concatenate([res.results[c]["out"] for c in range(NCORES)],
                         axis=0).astype(np.float32)
    if _DEBUG:
        kernel.dbg = np.concatenate(
            [res.results[c]["dbg"] for c in range(NCORES)], axis=0)
    return out

